# revision 1
# baseline (speedup 1.0000x reference)
"""Trainium2 Bass kernel for nn_ExpansionContrastModule.

Strategy: pure data parallel over 8 cores; each core processes half of one
batch image (128 of 256 rows), with a 3-row halo so the dilated contrast
convs and the 3x3 mas conv need no cross-core traffic.

Per-core layout: channels of the reduced tensor x (16) are packed 8 groups
to the 128 SBUF partitions (partition 16g+c = row-group g, channel c), so
all elementwise work runs at full 128-lane width.  Row-groups cover 8 rows
each; two sequential 64-row macro-halves per core bound SBUF usage.
"""
import sys
import ml_dtypes
import numpy as np

sys.path.insert(0, "/opt/trn_rl_repo")

import concourse.bass as bass
import concourse.bacc as bacc
import concourse.mybir as mybir
from concourse.tile import TileContext
from concourse.bass_utils import run_bass_kernel_spmd

F32 = mybir.dt.float32
AF = mybir.ActivationFunctionType
ALU = mybir.AluOpType

N_CORES = 8
C = 128        # input channels
CR = 16        # reduced channels
H = W = 256
CH = 128       # rows per core (half an image)
MH = 2         # macro-halves per core
HB = 64        # rows per macro-half
G = 8          # row-groups per macro-half
GR = 8         # rows per group
XR = GR + 6    # x tile rows (3-row halo each side)
XP = 4         # x tile left/right col pad (4 for bf16 4B alignment)
XW = W + 2 * XP  # x tile cols
NQ = 1         # slabs per macro-half
QR = 8         # group-rows per slab
QF = QR * W    # free elems per slab (2048)
BF = mybir.dt.bfloat16

BN_EPS = 1e-5

# scal column indices
S_W0, S_W1_4, S_W2, S_V0, S_V1_2, S_V2 = 0, 1, 2, 3, 4, 5
S_G0, S_G1, S_G2, S_BOUT, S_MB1, S_MW2, S_MB2 = 6, 7, 8, 9, 10, 11, 12

_CACHE = {}


def _fl(t):
    """Flatten the two free dims of a [P, a, b] tile AP into [P, a*b]."""
    return t[:, :, :].rearrange("p a b -> p (a b)")


def build_nc(loop_reps=0):
    nc = bacc.Bacc("TRN2", target_bir_lowering=False, debug=False,
                   num_devices=N_CORES)
    cen_p = nc.dram_tensor("cen", [C, CH + 6, W], F32, kind="ExternalInput")
    cen_b = nc.dram_tensor("cen_bf", [C, CH + 6, W], BF, kind="ExternalInput")
    mas_p = nc.dram_tensor("mas", [CH + 8, W], BF, kind="ExternalInput")
    cbf_d = nc.dram_tensor("cbf", [C, 2448], BF, kind="ExternalInput")
    cf32_d = nc.dram_tensor("cf32", [C, 18], F32, kind="ExternalInput")
    out_p = nc.dram_tensor("out", [C, CH, W], F32, kind="ExternalOutput")

    with TileContext(nc) as tc:
        import contextlib
        _stk = contextlib.ExitStack()
        with _stk:
            cpool = _stk.enter_context(tc.tile_pool(name="const", bufs=1))
            xpool = _stk.enter_context(tc.tile_pool(name="x", bufs=2))
            xopool = _stk.enter_context(tc.tile_pool(name="xo", bufs=1))
            bandpool = _stk.enter_context(tc.tile_pool(name="band", bufs=4))
            dpool = _stk.enter_context(tc.tile_pool(name="d", bufs=3))
            opool = _stk.enter_context(tc.tile_pool(name="o", bufs=4))
            trpool = _stk.enter_context(tc.tile_pool(name="tr", bufs=5))
            tspool = _stk.enter_context(tc.tile_pool(name="tsb", bufs=2))
            typool = _stk.enter_context(tc.tile_pool(name="ty", bufs=2))
            zpool = _stk.enter_context(tc.tile_pool(name="z", bufs=1))
            qpool = _stk.enter_context(tc.tile_pool(name="q8", bufs=5))
            g9pool = _stk.enter_context(tc.tile_pool(name="g9", bufs=2))
            mas9pool = _stk.enter_context(tc.tile_pool(name="m9", bufs=2))
            gbpool = _stk.enter_context(tc.tile_pool(name="gb", bufs=1))
            cfpool = _stk.enter_context(tc.tile_pool(name="cf", bufs=3))
            pspool = _stk.enter_context(tc.tile_pool(name="ps", bufs=2, space="PSUM"))
            pbpool = _stk.enter_context(tc.tile_pool(name="pb", bufs=2, space="PSUM"))
            pupool = _stk.enter_context(tc.tile_pool(name="pu", bufs=2, space="PSUM"))
            # ---- constants ----
            cbf_sb = cpool.tile([C, 2448], BF, tag="c_bf")
            cf32_sb = cpool.tile([C, 18], F32, tag="c_f32")
            nc.sync.dma_start(out=cbf_sb[:], in_=cbf_d[:])
            nc.sync.dma_start(out=cf32_sb[:], in_=cf32_d[:])

            def sc(col, p=C):
                return cf32_sb[0:p, 5 + col:6 + col]

            def emit_half(mh):
                base = mh * HB

                # ---- x conv: x[16g+c, j, XP+w] = w_in . cen(base+8g+j-3)
                # x is bf16 so the contrast stage runs DVE 2x_1P mode; x_odd
                # is x shifted left one column so odd-dw reads stay 4B-aligned
                x = xpool.tile([C, XR, XW], BF, tag="x")
                nc.gpsimd.memset(x[:, :, 0:XP], 0.0)
                nc.gpsimd.memset(x[:, :, W + XP:XW], 0.0)

                cen_g = cen_b[:, base + 3: base + 67, :].rearrange(
                    "c (g x) w -> c g x w", x=GR)
                # image-edge halo rows for group 0 (top) and group 7 (bottom):
                # cen_p rows base+0..3 and base+67..70 (zero-padded at edges)
                etop = bandpool.tile([C, 3, W], BF, tag="edge")
                nc.sync.dma_start(out=etop[:], in_=cen_b[:, base:base + 3, :])
                ebot = bandpool.tile([C, 3, W], BF, tag="edge")
                nc.sync.dma_start(out=ebot[:],
                                  in_=cen_b[:, base + 67:base + 70, :])
                for r in range(GR):
                    band = bandpool.tile([C, G, W], BF, tag="band")
                    nc.sync.dma_start(out=band[:], in_=cen_g[:, :, r, :])
                    pxc = pspool.tile([C, W], F32, tag="ps")
                    for g in range(G):
                        nc.tensor.matmul(
                            pxc[:], cbf_sb[:, g * C:(g + 1) * C], band[:, g, :],
                            start=(g == 0), stop=(g == G - 1))
                    nc.scalar.activation(
                        x[:, 3 + r, XP: XP + W], pxc[:], AF.Identity,
                        bias=cf32_sb[:, 0:1], scale=1.0)

                for j in range(3):
                    pe = pspool.tile([C, W], F32, tag="ps")
                    nc.tensor.matmul(pe[:], cbf_sb[:, 0:C],
                                     etop[:, j, :], start=True,
                                     stop=True)
                    bt_c = 1 if mh == 0 else 0
                    nc.scalar.activation(
                        x[0:CR, j, XP: XP + W], pe[0:CR, :], AF.Identity,
                        bias=cf32_sb[0:CR, bt_c:bt_c + 1], scale=1.0)
                    pe2 = pspool.tile([C, W], F32, tag="ps")
                    nc.tensor.matmul(pe2[:], cbf_sb[:, (G - 1) * C:G * C],
                                     ebot[:, j, :], start=True,
                                     stop=True)
                    # start partition must be a multiple of 32; rows 96:112 get
                    # junk here and are re-written by the interior-halo DMA
                    # below (WAW-ordered by Tile).
                    bb_c = 2 if mh == MH - 1 else 0
                    nc.scalar.activation(
                        x[96: C, 11 + j, XP: XP + W], pe2[96: C, :],
                        AF.Identity, bias=cf32_sb[96:C, bb_c:bb_c + 1], scale=1.0)

                # interior halos between groups via partition-shifted SBUF DMA
                nc.sync.dma_start(out=x[CR:C, 0:3, XP:XP + W],
                                  in_=x[0:C - CR, GR:GR + 3, XP:XP + W])
                nc.sync.dma_start(out=x[0:C - CR, GR + 3:GR + 6, XP:XP + W],
                                  in_=x[CR:C, 3:6, XP:XP + W])

                # ---- mas path ----
                mas9 = mas9pool.tile([72, QR, W], BF, tag="m9")
                nc.gpsimd.memset(mas9[:, :, 0:1], 0.0)
                nc.gpsimd.memset(mas9[:, :, W - 1:W], 0.0)
                t = 0
                for dy in (-1, 0, 1):
                    for dx in (-1, 0, 1):
                        cs, cd = max(0, dx), max(0, -dx)
                        n = W - abs(dx)
                        msrc = mas_p[base + dy + 1:
                                 base + dy + 1 + 64, :]
                        msrc = msrc.rearrange("(g x) w -> g x w", x=GR)
                        nc.sync.dma_start(
                            out=mas9[G * t:G * (t + 1), :, cd:cd + n],
                            in_=msrc[:, 0:QR, cs:cs + n])
                        t += 1
                msil = qpool.tile([G, QF], BF, tag="q8")
                mm_q = qpool.tile([G, QF], BF, tag="q8")
                m9f = _fl(mas9)
                for c2 in range(4):
                    cs2 = slice(512 * c2, 512 * (c2 + 1))
                    pm = pupool.tile([G, 512], F32, tag="pu")
                    nc.tensor.matmul(pm[:], cbf_sb[0:72, 1416:1424], m9f[:, cs2],
                                 start=True, stop=True)
                    m_t = mas9pool.tile([G, 512], BF, tag="qs")
                    nc.scalar.activation(m_t[:], pm[:], AF.Identity,
                                     bias=sc(S_MB1, G), scale=1.0)
                    m_s = mas9pool.tile([G, 512], BF, tag="qs")
                    nc.scalar.activation(m_s[:], m_t[:], AF.Sigmoid)
                    nc.vector.tensor_tensor(msil[:, cs2], m_t[:], m_s[:],
                                        ALU.mult)
                    nc.scalar.activation(mm_q[:, cs2], msil[:, cs2],
                                     AF.Sigmoid, bias=sc(S_MB2, G),
                                     scale=sc(S_MW2, G))


                # x_odd[c] = x[c+1], one flat shifted copy (pads included)
                x_odd = xopool.tile([C, XR, XW], BF, tag="xo")
                xf = _fl(x)
                xof = _fl(x_odd)
                nc.scalar.copy(xof[:, 0:XR * XW - 1], xf[:, 1:XR * XW])

                j0 = 3

                def xin(dh, dw):
                    if dw % 2 == 0:
                        return x[:, j0 + dh:j0 + QR + dh,
                                 XP + dw:XP + W + dw]
                    return x_odd[:, j0 + dh:j0 + QR + dh,
                                 XP + dw - 1:XP + W + dw - 1]

                if True:
                    # ---- contrast stage ----
                    ts_tiles = []
                    for s in (1, 3):
                        dirs = [(-s, -s), (-s, 0), (-s, s), (0, -s)]
                        o_t = []
                        for (dh, dw) in dirs:
                            d1 = dpool.tile([C, QR, W], BF, tag="d")
                            nc.vector.tensor_tensor(
                                d1[:], x[:, j0:j0 + QR, XP:XP + W],
                                xin(dh, dw), ALU.subtract)
                            d2 = dpool.tile([C, QR, W], BF, tag="d")
                            nc.vector.tensor_tensor(
                                d2[:], x[:, j0:j0 + QR, XP:XP + W],
                                xin(-dh, -dw), ALU.subtract)
                            o = opool.tile([C, QR, W], BF, tag="o")
                            nc.vector.tensor_tensor(_fl(o), _fl(d1), _fl(d2),
                                                    ALU.mult)
                            o_t.append(o)

                        def tree(op, eng):
                            a = trpool.tile([C, QF], BF, tag="tr")
                            eng.tensor_tensor(a[:], _fl(o_t[0]),
                                              _fl(o_t[1]), op)
                            b = trpool.tile([C, QF], BF, tag="tr")
                            eng.tensor_tensor(b[:], _fl(o_t[2]),
                                              _fl(o_t[3]), op)
                            r_ = trpool.tile([C, QF], BF, tag="tr")
                            nc.vector.tensor_tensor(r_[:], a[:], b[:], op)
                            return r_

                        min4 = tree(ALU.min, nc.vector)
                        max4 = tree(ALU.max, nc.vector)
                        sum4 = tree(ALU.add, nc.vector)
                        q1 = trpool.tile([C, QF], BF, tag="tr")
                        nc.vector.tensor_scalar_mul(q1[:], sum4[:], sc(S_W1_4))
                        qa = trpool.tile([C, QF], BF, tag="tr")
                        nc.vector.tensor_scalar_mul(qa[:], min4[:], sc(S_W0))
                        qb = trpool.tile([C, QF], BF, tag="tr")
                        nc.vector.tensor_scalar_mul(qb[:], max4[:], sc(S_W2))
                        qu = trpool.tile([C, QF], BF, tag="tr")
                        nc.vector.tensor_tensor(qu[:], qa[:], qb[:], ALU.add)
                        t_s = tspool.tile([C, QF], BF, tag="ts")
                        nc.vector.tensor_tensor(t_s[:], qu[:], q1[:], ALU.add)
                        ts_tiles.append(t_s)

                    t1, t3 = ts_tiles
                    mn = typool.tile([C, QF], BF, tag="tymn")
                    nc.vector.tensor_tensor(mn[:], t1[:], t3[:], ALU.min)
                    mx = typool.tile([C, QF], BF, tag="tymx")
                    nc.vector.tensor_tensor(mx[:], t1[:], t3[:], ALU.max)
                    sm = typool.tile([C, QF], BF, tag="tysm")
                    nc.vector.tensor_tensor(sm[:], t1[:], t3[:], ALU.add)

                    # ---- bc 1x1 conv + BN + SiLU (bf16 z path) ----
                    # the scale2-weighted min/mean/max combine is folded into
                    # three accumulating matmuls with pre-scaled bc weights
                    t_bn = zpool.tile([C, QF], BF, tag="tbn")
                    for h2 in range(2):
                        pbc = pbpool.tile([C, 1024], F32, tag="pb")
                        for c2 in range(2):
                            lo = 1024 * h2 + 512 * c2
                            cc = slice(512 * c2, 512 * (c2 + 1))
                            nc.tensor.matmul(pbc[:, cc],
                                             cbf_sb[:, 1024:1152], mn[:, lo:lo + 512],
                                             start=True, stop=False)
                            nc.tensor.matmul(pbc[:, cc],
                                             cbf_sb[:, 1152:1280], sm[:, lo:lo + 512],
                                             start=False, stop=False)
                            nc.tensor.matmul(pbc[:, cc],
                                             cbf_sb[:, 1280:1408], mx[:, lo:lo + 512],
                                             start=False, stop=True)
                        nc.scalar.activation(t_bn[:, 1024 * h2:1024 * (h2 + 1)],
                                             pbc[:], AF.Identity,
                                             bias=cf32_sb[:, 4:5],
                                             scale=cf32_sb[:, 3:4])
                    sg = zpool.tile([C, QF], BF, tag="sg")
                    nc.scalar.activation(sg[:], t_bn[:], AF.Sigmoid)
                    z_q = zpool.tile([C, QF], BF, tag="z")
                    nc.vector.tensor_tensor(z_q[:], t_bn[:], sg[:], ALU.mult)

                    # ---- w_out 1x1 + sigmoid -> om ----
                    om_q = qpool.tile([G, QF], BF, tag="q8")
                    for c2 in range(4):
                        pu = pupool.tile([G, 512], F32, tag="pu")
                        nc.tensor.matmul(pu[:], cbf_sb[:, 1408:1416],
                                         z_q[:, 512 * c2:512 * (c2 + 1)],
                                         start=True, stop=True)
                        nc.scalar.activation(om_q[:, 512 * c2:512 * (c2 + 1)],
                                             pu[:], AF.Sigmoid,
                                             bias=sc(S_BOUT, G), scale=1.0)

                    # ---- gate = s0*om + s1*mm + s2*om*mm + s3 ----
                    bt = qpool.tile([G, QF], BF, tag="q8")
                    nc.vector.tensor_scalar(bt[:], mm_q[:], sc(S_G2, G),
                                            sc(S_G0, G), ALU.mult, ALU.add)
                    ct = qpool.tile([G, QF], BF, tag="q8")
                    nc.vector.tensor_tensor(ct[:], om_q[:], bt[:], ALU.mult)
                    ge = qpool.tile([G, QF], BF, tag="q8")
                    nc.vector.tensor_scalar_mul(ge[:], mm_q[:], sc(S_G1, G))
                    gate9 = g9pool.tile([9, QF], BF, tag="g9")
                    nc.gpsimd.memset(gate9[:], 1.0)
                    nc.vector.tensor_tensor(gate9[0:G, :], ct[:], ge[:],
                                            ALU.add)

                    # ---- broadcast gate + final multiply ----
                    # gate broadcast lands in PSUM; stage to SBUF (ACT) so
                    # the multiply itself can run on the idle Pool engine
                    for g in range(G):
                        r0 = base + GR * g
                        cf = cfpool.tile([C, GR, W], F32, tag="cf")
                        nc.sync.dma_start(
                            out=cf[:], in_=cen_p[:, r0 + 3:r0 + 3 + GR, :])
                        for h2 in range(2):
                            pg = pbpool.tile([C, 1024], F32, tag="pb")
                            for c2 in range(2):
                                lo = 1024 * h2 + 512 * c2
                                nc.tensor.matmul(
                                    pg[:, 512 * c2:512 * (c2 + 1)],
                                    cbf_sb[0:9, 1424 + g * C:1424 + (g + 1) * C],
                                    gate9[:, lo:lo + 512],
                                    start=True, stop=True)
                            cfs = cf[:, 4 * h2:4 * h2 + 4, :].rearrange(
                                "p a b -> p (a b)")
                            if mh == MH - 1 or (2 * g + h2) % 8 < 3:
                                nc.vector.tensor_tensor(cfs, cfs, pg[:],
                                                        ALU.mult)
                            else:
                                gb = gbpool.tile([C, 1024], F32, tag="gb")
                                nc.scalar.copy(gb[:], pg[:])
                                nc.gpsimd.tensor_tensor(cfs, cfs, gb[:],
                                                        ALU.mult)
                        nc.sync.dma_start(out=out_p[:, r0:r0 + GR, :],
                                          in_=cf[:])

            rep_ctx = (tc.For_i(0, loop_reps, 1) if loop_reps
                       else contextlib.nullcontext())
            with rep_ctx:
                for mh in range(MH):
                    emit_half(mh)
    nc.compile()
    return nc


def _softmax(v):
    e = np.exp(v - v.max())
    return e / e.sum()


def _prep_consts(inp):
    w = _softmax(inp['scale1'])
    v = _softmax(inp['scale2'])
    s3 = _softmax(inp['scale3'])
    inv = inp['bn_gamma'] / np.sqrt(inp['bn_var'] + BN_EPS)
    bnb = inp['bn_beta'] - inp['bn_mean'] * inv

    consts = {}
    w_in_blk = np.zeros((C, G, C), np.float32)
    for g in range(G):
        w_in_blk[:, g, CR * g:CR * (g + 1)] = inp['w_in'].T
    consts['w_in_blk'] = w_in_blk
    consts['b_in_t'] = np.tile(inp['b_in'], G)[:, None].astype(np.float32)
    bc_blk = np.kron(np.eye(G), inp['bc_w'].T)
    bc3 = np.stack([v[0] * bc_blk, (v[1] / 2.0) * bc_blk, v[2] * bc_blk], 1)
    consts['bc_lhsT'] = bc3.astype(np.float32)
    consts['bn_s'] = np.tile(inv, G)[:, None].astype(np.float32)
    consts['bn_b'] = np.tile(bnb, G)[:, None].astype(np.float32)
    consts['wout_lhsT'] = np.kron(np.eye(G), inp['w_out'][0][:, None]).astype(
        np.float32)
    k_flat = inp['mas_w1'][0, 0].reshape(9)
    consts['mas_lhsT'] = np.kron(k_flat[:, None], np.eye(G)).astype(np.float32)
    bcast = np.zeros((9, G, C), np.float32)
    for g in range(G):
        bcast[g, g, :] = 1.0
    bcast[8, :, :] = s3[3]
    consts['bcast_lhsT'] = bcast
    scal = np.zeros((C, 13), np.float32)
    vals = [w[0], w[1] / 4.0, w[2], v[0], v[1] / 2.0, v[2],
            s3[0], s3[1], s3[2], inp['b_out'][0], inp['mas_b1'][0],
            inp['mas_w2'][0, 0], inp['mas_b2'][0]]
    scal[:] = np.asarray(vals, np.float32)[None, :]
    cbf = np.zeros((C, 2448), np.float32)
    cbf[:, 0:1024] = consts['w_in_blk'].reshape(C, G * C)
    cbf[:, 1024:1408] = consts['bc_lhsT'].reshape(C, 3 * C)
    cbf[:, 1408:1416] = consts['wout_lhsT']
    cbf[0:72, 1416:1424] = consts['mas_lhsT']
    cbf[0:9, 1424:2448] = consts['bcast_lhsT'].reshape(9, G * C)
    cf32 = np.zeros((C, 18), np.float32)
    cf32[:, 0:1] = consts['b_in_t']
    cf32[:, 3:4] = consts['bn_s']
    cf32[:, 4:5] = consts['bn_b']
    cf32[:, 5:18] = scal
    return {'cbf': cbf.astype(ml_dtypes.bfloat16), 'cf32': cf32,
            'b_in_t': consts['b_in_t']}


def run(inputs, trace=False):
    inp = {k: np.asarray(v) for k, v in inputs.items()}
    B = inp['cen'].shape[0]
    consts = _prep_consts(inp)

    if 'nc' not in _CACHE:
        _CACHE['nc'] = build_nc()
    nc = _CACHE['nc']

    in_maps = []
    for core in range(N_CORES):
        b, hf = core // 2, core % 2
        r0 = CH * hf
        cen_pad = np.pad(inp['cen'][b], ((0, 0), (3, 3), (0, 0)))
        mas_pad = np.pad(inp['mas'][b, 0], ((1, 9), (0, 0)))
        cen_core = np.ascontiguousarray(cen_pad[:, r0:r0 + CH + 6, :])
        cf32 = consts['cf32'].copy()
        if hf != 0:
            cf32[:, 1:2] = consts['b_in_t']
        if hf != 1:
            cf32[:, 2:3] = consts['b_in_t']
        m = {
            'cen': cen_core,
            'cen_bf': cen_core.astype(ml_dtypes.bfloat16),
            'mas': np.ascontiguousarray(
                mas_pad[r0:r0 + CH + 8, :]).astype(ml_dtypes.bfloat16),
            'cbf': consts['cbf'],
            'cf32': cf32,
        }
        in_maps.append(m)

    res = run_bass_kernel_spmd(nc, in_maps, list(range(N_CORES)), trace=trace)

    out = np.empty_like(inp['cen'])
    for core in range(N_CORES):
        b, hf = core // 2, core % 2
        out[b, :, CH * hf:CH * (hf + 1), :] = res.results[core]['out']
    return out, res


def kernel(**inputs):
    return run(inputs)[0]


def bench(inputs, iters=30, reps=0):
    """Time repeated executions with device-resident inputs (no donation).

    Returns (out, per_call_seconds_list). The kernel writes every output
    element, so dropping the zero-buffer donation is safe.
    """
    import time
    import jax
    import jax.numpy as jnp
    from jax.sharding import Mesh, PartitionSpec
    from jax.experimental.shard_map import shard_map
    from concourse import bass2jax
    from concourse import mybir as mb

    inp = {k: np.asarray(v) for k, v in inputs.items()}
    consts = _prep_consts(inp)
    key = ('nc', reps)
    if key not in _CACHE:
        _CACHE[key] = build_nc(loop_reps=reps)
    nc = _CACHE[key]

    in_maps = []
    for core in range(N_CORES):
        b, hf = core // 2, core % 2
        r0 = CH * hf
        cen_pad = np.pad(inp['cen'][b], ((0, 0), (3, 3), (0, 0)))
        mas_pad = np.pad(inp['mas'][b, 0], ((1, 9), (0, 0)))
        cen_core = np.ascontiguousarray(cen_pad[:, r0:r0 + CH + 6, :])
        cf32 = consts['cf32'].copy()
        if hf != 0:
            cf32[:, 1:2] = consts['b_in_t']
        if hf != 1:
            cf32[:, 2:3] = consts['b_in_t']
        m = {
            'cen': cen_core,
            'cen_bf': cen_core.astype(ml_dtypes.bfloat16),
            'mas': np.ascontiguousarray(
                mas_pad[r0:r0 + CH + 8, :]).astype(ml_dtypes.bfloat16),
            'cbf': consts['cbf'],
            'cf32': cf32,
        }
        in_maps.append(m)

    bass2jax.install_neuronx_cc_hook()
    in_names, out_names, out_avals, zero_outs = [], [], [], []
    for alloc in nc.m.functions[0].allocations:
        if not isinstance(mybir.MemoryLocationSet, type) or not isinstance(
                alloc, mybir.MemoryLocationSet):
            continue
        name = alloc.memorylocations[0].name
        pname = (nc.partition_id_tensor.name if nc.partition_id_tensor
                 else None)
        if alloc.kind == "ExternalInput":
            if name != pname:
                in_names.append(name)
        elif alloc.kind == "ExternalOutput":
            out_names.append(name)
            out_avals.append(jax.core.ShapedArray(
                tuple(alloc.tensor_shape), mybir.dt.np(alloc.dtype)))
            zero_outs.append(np.zeros(tuple(alloc.tensor_shape),
                                      mybir.dt.np(alloc.dtype)))
    n_params = len(in_names)
    all_names = in_names + out_names
    if nc.partition_id_tensor:
        all_names = all_names + [nc.partition_id_tensor.name]

    def _body(*args):
        operands = list(args)
        if nc.partition_id_tensor:
            operands.append(bass2jax.partition_id_tensor())
        outs = bass2jax._bass_exec_p.bind(
            *operands,
            out_avals=tuple(out_avals),
            in_names=tuple(all_names),
            out_names=tuple(out_names),
            lowering_input_output_aliases=(),
            sim_require_finite=True,
            sim_require_nnan=True,
            nc=nc,
        )
        return tuple(outs)

    devices = jax.devices()[:N_CORES]
    mesh = Mesh(np.asarray(devices), ("core",))
    nin = n_params + len(out_names)
    sharded = jax.jit(
        shard_map(_body, mesh=mesh,
                  in_specs=(PartitionSpec("core"),) * nin,
                  out_specs=(PartitionSpec("core"),) * len(out_names),
                  check_rep=False),
        donate_argnums=tuple(range(n_params, n_params + len(out_names))),
        keep_unused=True,
    )
    concat_in = [np.concatenate([in_maps[c][nm] for c in range(N_CORES)], 0)
                 for nm in in_names]
    concat_zero = [np.zeros((N_CORES * z.shape[0], *z.shape[1:]), z.dtype)
                   for z in zero_outs]
    sh = jax.sharding.NamedSharding(mesh, PartitionSpec("core"))
    dev_in = [jax.device_put(a, sh) for a in concat_in]
    prev = jax.device_put(concat_zero[0], sh)

    outs = sharded(*dev_in, prev)
    jax.block_until_ready(outs)
    result = np.asarray(outs[0]).copy()
    prev = outs[0]
    times = []
    for _ in range(iters):
        t0 = time.perf_counter()
        outs = sharded(*dev_in, prev)
        jax.block_until_ready(outs)
        times.append(time.perf_counter() - t0)
        prev = outs[0]

    full = np.empty((4, C, H, W), np.float32)
    arr = result.reshape(N_CORES, C, CH, W)
    for core in range(N_CORES):
        b, hf = core // 2, core % 2
        full[b, :, CH * hf:CH * (hf + 1), :] = arr[core]
    return full, times

